# revision 41
# baseline (speedup 1.0000x reference)
"""Linear (kernel-feature) attention for Trainium2, sharded over 8 NeuronCores.

Problem: B=4, H=16, S=4096, D=64 fp32.
    phi(x) = elu(x) + 1 = exp(min(x,0)) + relu(x)
    kv   = phi_k_masked^T @ V          [d, v]
    k1   = phi_k^T @ mask              [d]
    out  = (phi_q @ kv) / (phi_q @ k1 + eps)

Sharding: 64 (b,h) slices -> 8 per core, processed as 4 pairs.
All device data is bf16 (tolerance 2e-2; measured end-to-end error of this
pipeline is ~3.3e-3), halving the HBM roofline to ~16 MB/core and doubling
PE stream rate + enabling FWL weight loads.

Host-side layout (part of sharding, costs no HW time). One fused input
tensor per pair for a single ~3.1 MB DMA (best HBM efficiency):
  inp[pair] = [128, 12352] bf16, cols:
    [0, 4096)      kp: per 128-row n-tile t, cols t*128+{0:64,64:128} =
                   slice {a,b}'s K tile [n-part, d]
    [4096, 8256)   vx: per tile, cols 4096+t*130+{0:65,65:130} = slice
                   {a,b}'s [V | mask] (mask col fuses k1 into the kv
                   matmul; mask also folded into V on host if not all-1)
    [8256, 12352)  qT: partitions 0:64 = slice a's q^T [d, n], 64:128 = b's
  outc[pair] = [128, 2*FREE] bf16, natural [n-part, t, d] per slice.

Device per pair (emitted as a 2-stage software pipeline so no engine FIFO
head-of-line-blocks on a cross-engine dependency):
  phi_k is never materialized: phi_k = exp(min(k,0)) + k + (-min(k,0)), so
  M1 accumulates THREE weight chains into one psum group - raw k (zero
  engine deps, starts at DMA landing), xm = -min(k,0) (one fused DVE op),
  exp(-xm) (ACT) - 96 matmuls [128,128]x[128,130] -> kv_ext psum
  [128,130]; valid quadrants [0:64,0:65]=slice a, [64:128,65:130]=slice b;
  garbage quadrants are zeroed in the bf16 copy so M2 can run full-K.
  phi_q = exp(0.125*min(q,0)) [DVE min + ACT exp] + relu(0.125*q) [ACT;
  DVE for pair 0], combined by a DVE add (ACT and DVE are the
  co-bottlenecks; this split balances them at ~11us/pair each).
  M2: per slice, one matmul per n-tile with fused [kv|k1] rhs (N=65) and
  full-width [128,128] pq weights (other slice's quadrant is zero; 128
  weight columns enable FWL, halving LDWEIGHTS) into 7-tile psum groups
  at 512-col offsets of a 3-bank + 2-bank psum tile pair; DVE reciprocal
  batched over the strided nrm columns (eps dropped: nrm ~ 1e5 >> 1e-6),
  then batched broadcast multiplies evacuate psum -> bf16 out tile;
  per-slice output DMA.
"""

import sys

sys.path.insert(0, "/opt/trn_rl_repo")

import numpy as np
import ml_dtypes

B, H, S, D = 4, 16, 4096, 64
N_CORES = 8
SL = (B * H) // N_CORES  # slices per core = 8
PAIRS = SL // 2  # 4
NT = S // 128  # 32 n-tiles per slice
FREE = NT * D  # 2048 free cols per slice of output
KW = NT * 128  # 4096: packed-K region width
VW = NT * 130  # 4160: packed-[V|m] region width
INW = KW + VW + S  # 12352: fused input width
BF16 = ml_dtypes.bfloat16

_programs: dict = {}


def _build_program():
    from contextlib import ExitStack

    import concourse.bacc as bacc
    import concourse.tile as tile
    from concourse import mybir

    f32 = mybir.dt.float32
    bf16 = mybir.dt.bfloat16
    Alu = mybir.AluOpType
    Act = mybir.ActivationFunctionType

    nc = bacc.Bacc("TRN2", target_bir_lowering=False, debug=False)
    inp = nc.dram_tensor("inp", [PAIRS, 128, INW], bf16, kind="ExternalInput").ap()
    outc = nc.dram_tensor(
        "outc", [PAIRS, 128, 2 * FREE], bf16, kind="ExternalOutput"
    ).ap()

    # M2 tile groups: psum bank holds 7 tiles of 65 cols (455 <= 512)
    GROUPS = [(0, 7), (7, 7), (14, 7), (21, 7), (28, 4)]

    with tile.TileContext(nc) as tc, ExitStack() as ctx:
        ipool = ctx.enter_context(tc.tile_pool(name="ipool", bufs=4))
        tmp = ctx.enter_context(tc.tile_pool(name="tmp", bufs=3))
        pqp = ctx.enter_context(tc.tile_pool(name="pqp", bufs=2))
        kvp = ctx.enter_context(tc.tile_pool(name="kvp", bufs=2))
        nrmp = ctx.enter_context(tc.tile_pool(name="nrmp", bufs=4))
        outp = ctx.enter_context(tc.tile_pool(name="outp", bufs=3))
        ps_kv = ctx.enter_context(tc.tile_pool(name="ps_kv", bufs=2, space="PSUM"))
        # M2 psum: 3-bank + 2-bank tiles (groups at 512-col offsets) so the
        # nrm reciprocals and the evac multiply batch across groups.
        ps_a = ctx.enter_context(tc.tile_pool(name="ps_a", bufs=1, space="PSUM"))
        ps_b = ctx.enter_context(tc.tile_pool(name="ps_b", bufs=1, space="PSUM"))

        def front_half(pair):
            """DMA in, phi_k, M1, kv evac, phi_q. Returns (kv_sb, pq)."""
            it = ipool.tile([128, INW], bf16)
            kt = it[:, 0:KW]
            vt = it[:, KW : KW + VW]
            qt = it[:, KW + VW : INW]
            # 3-way split (range deps): phi_k needs only k, M1 adds v,
            # phi_q waits for q - each stage starts as early as possible.
            # (Measured dead ends: forcing arrival order via SWDGE, the
            # Scalar HWDGE queue, DMA-completion chaining, or compute-
            # anchored staggering all LOSE to the default concurrent
            # transfers - concurrency hides the ~2us completion latency.)
            nc.sync.dma_start(out=it[:, 0:KW], in_=inp[pair][:, 0:KW])
            nc.sync.dma_start(out=it[:, KW : KW + VW], in_=inp[pair][:, KW : KW + VW])
            nc.sync.dma_start(out=it[:, KW + VW : INW], in_=inp[pair][:, KW + VW : INW])

            # ---- phi_k pieces. phi_k = exp(min(k,0)) + relu(k)
            #                          = exp(min(k,0)) + k + (-min(k,0)),
            # so M1 accumulates THREE weight chains: raw k (no engine dep
            # at all - starts at DMA landing), xm = -min(k,0) (one fused
            # DVE op), and exp(-xm) (ACT). bf16 x bf16 products are exact
            # in fp32, so the k + (-min) cancellation is exact.
            xm = tmp.tile([128, S], bf16, tag="r")
            nc.vector.tensor_scalar(xm, kt, 0.0, -1.0, Alu.min, Alu.mult)
            ek = tmp.tile([128, S], bf16, tag="e")
            nc.scalar.activation(ek, xm, Act.Exp, scale=-1.0)

            kv_ps = ps_kv.tile([128, 130], f32)
            for src in (kt, xm, ek):
                for t in range(NT):
                    nc.tensor.matmul(
                        kv_ps,
                        src[:, t * 128 : (t + 1) * 128],
                        vt[:, t * 130 : (t + 1) * 130],
                        start=(src is kt and t == 0),
                        stop=(src is ek and t == NT - 1),
                    )
            # zero the garbage quadrants so M2 can use full-K (128) weights:
            # the wrong slice's pq half multiplies zeros. Full-width weights
            # also enable FWL (halves M2 LDWEIGHTS time).
            kv_sb = kvp.tile([128, 130], bf16)
            nc.vector.memset(kv_sb, 0.0)
            nc.scalar.copy(kv_sb[0:64, 0:65], kv_ps[0:64, 0:65])
            nc.scalar.copy(kv_sb[64:128, 65:130], kv_ps[64:128, 65:130])

            # ---- phi_q = exp(0.125*min(q,0)) + 0.125*relu(q), into pq
            eq = tmp.tile([128, S], bf16, tag="e")
            nc.vector.tensor_scalar_min(eq, qt, 0.0)
            nc.scalar.activation(eq, eq, Act.Exp, scale=0.125)
            rq = tmp.tile([128, S], bf16, tag="r")
            if pair in (0, PAIRS - 1):
                # pair 0: DVE has slack from the 3-chain M1 (head).
                # last pair: keeps its relu off the END of the ACT wall, so
                # the tail chain (add -> M2 -> divide -> store) starts as
                # soon as exp_k finishes (shorter critical tail).
                nc.vector.tensor_scalar(rq, qt, 0.125, 0.0, Alu.mult, Alu.max)
            else:
                nc.scalar.activation(rq, qt, Act.Relu, scale=0.125)
            pq = pqp.tile([128, S], bf16)
            # two halves: M2's first groups start while the second half runs
            nc.vector.tensor_tensor(
                pq[:, 0 : S // 2], eq[:, 0 : S // 2], rq[:, 0 : S // 2], Alu.add
            )
            nc.vector.tensor_tensor(
                pq[:, S // 2 : S], eq[:, S // 2 : S], rq[:, S // 2 : S], Alu.add
            )
            return kv_sb, pq

        def back_half(pair, kv_sb, pq):
            """M2, divide, store for a pair whose front half was emitted."""
            out_sb = outp.tile([128, 2 * FREE], bf16)
            for rr in range(2):
                # [kv | k1] for this slice; other slice's quadrant is zero,
                # so full-K (128) pq weights are safe and FWL-eligible
                rhs = kv_sb[:, 0:65] if rr == 0 else kv_sb[:, 65:130]
                base = rr * FREE
                poa = ps_a.tile([128, 1536], f32)  # groups 0-2 at 512-col offsets
                pob = ps_b.tile([128, 1024], f32)  # groups 3-4
                for gi, (g0, gs) in enumerate(GROUPS):
                    po = poa if gi < 3 else pob
                    off = 512 * (gi if gi < 3 else gi - 3)
                    for i in range(gs):
                        t = g0 + i
                        nc.tensor.matmul(
                            po[:, off + i * 65 : off + (i + 1) * 65],
                            pq[:, t * 128 : (t + 1) * 128],
                            rhs,
                            start=(i == 0),
                            stop=(i == gs - 1),
                            skip_group_check=True,
                        )
                # groups 0-2: one batched reciprocal + one 4D-AP evac multiply
                pa4 = (
                    poa.rearrange("p (g x) -> p g x", x=512)[:, :, 0:455]
                    .rearrange("p g (i c) -> p g i c", c=65)
                )
                nsa = nrmp.tile([128, 21], f32, tag="nsa")
                nsa3 = nsa.rearrange("p (g i) -> p g i", g=3)
                nc.vector.reciprocal(
                    nsa.rearrange("p (g i o) -> p g i o", g=3, o=1),
                    pa4[:, :, :, 64:65],
                )
                nc.vector.tensor_tensor(
                    out_sb[:, base : base + 1344].rearrange(
                        "p (g i c) -> p g i c", g=3, c=64
                    ),
                    pa4[:, :, :, 0:64],
                    nsa3.broadcast_to([128, 3, 7, 64]),
                    Alu.mult,
                )
                # groups 3-4: one batched reciprocal (tail lanes are don't-care),
                # per-group evac multiplies
                pb4 = (
                    pob.rearrange("p (g x) -> p g x", x=512)[:, :, 0:455]
                    .rearrange("p g (i c) -> p g i c", c=65)
                )
                nsb = nrmp.tile([128, 14], f32, tag="nsb")
                nsb3 = nsb.rearrange("p (g i) -> p g i", g=2)
                for bi, (g0, gs) in enumerate(GROUPS[3:]):
                    nc.vector.reciprocal(
                        nsb[:, bi * 7 : bi * 7 + gs].rearrange(
                            "p (i o) -> p i o", o=1
                        ),
                        pb4[:, bi, 0:gs, 64:65],
                    )
                for bi, (g0, gs) in enumerate(GROUPS[3:]):
                    nc.vector.tensor_tensor(
                        out_sb[:, base + g0 * 64 : base + (g0 + gs) * 64].rearrange(
                            "p (i c) -> p i c", c=64
                        ),
                        pb4[:, bi, 0:gs, 0:64],
                        nsb3[:, bi, 0:gs].broadcast_to([128, gs, 64]),
                        Alu.mult,
                    )
                # store per slice: slice a's DMA overlaps slice b's M2+divide
                nc.sync.dma_start(
                    out=outc[pair][:, base : base + FREE],
                    in_=out_sb[:, base : base + FREE],
                )

        # Two-stage software pipeline: emit pair p+1's front half before
        # pair p's back half so no engine FIFO head-of-line-blocks on a
        # cross-engine dependency (ready work is always ahead in the queue).
        state = front_half(0)
        for pair in range(1, PAIRS):
            nstate = front_half(pair)
            back_half(pair - 1, *state)
            state = nstate
        back_half(PAIRS - 1, *state)

    nc.compile()
    return nc


def _get_program():
    if "p" not in _programs:
        _programs["p"] = _build_program()
    return _programs["p"]


def _pack_inputs(query, key, value, attention_mask):
    """Shard + lay out inputs for the 8 cores (all bf16, fused per pair)."""
    q4 = np.asarray(query, dtype=np.float32).reshape(B * H, S, D)
    k4 = np.asarray(key, dtype=np.float32).reshape(B * H, S, D)
    v4 = np.asarray(value, dtype=np.float32).reshape(B * H, S, D)
    am = np.asarray(attention_mask, dtype=np.float32)

    inp = np.empty((N_CORES, PAIRS, 128, INW), dtype=BF16)
    # kp: [g, n, d] -> [core, pair, p, t*128 + s*64 + d]
    k6 = k4.reshape(N_CORES, PAIRS, 2, NT, 128, D)
    inp[:, :, :, 0:KW] = (
        k6.transpose(0, 1, 4, 3, 2, 5).reshape(N_CORES, PAIRS, 128, KW).astype(BF16)
    )
    # vx: [V*mask | mask] -> [core, pair, p, KW + t*130 + s*65 + c]
    mrow = np.repeat(am, H, axis=0).reshape(B * H, S, 1)  # [g, n, 1]
    if np.all(am == 1.0):
        vext = np.concatenate([v4, mrow], axis=-1)
    else:
        vext = np.concatenate([v4 * mrow, mrow], axis=-1)
    v6 = vext.reshape(N_CORES, PAIRS, 2, NT, 128, D + 1)
    inp[:, :, :, KW : KW + VW] = (
        v6.transpose(0, 1, 4, 3, 2, 5).reshape(N_CORES, PAIRS, 128, VW).astype(BF16)
    )
    # qT: [g, d, n] -> [core, pair, 2*64 d, n]
    inp[:, :, :, KW + VW : INW] = (
        np.ascontiguousarray(q4.transpose(0, 2, 1))
        .reshape(N_CORES, PAIRS, 2 * D, S)
        .astype(BF16)
    )
    return [{"inp": inp[c]} for c in range(N_CORES)]


def _unpack_output(results):
    outs = np.stack([r["outc"] for r in results])  # [cores, PAIRS, 128, 2*FREE]
    outs = outs.astype(np.float32).reshape(N_CORES, PAIRS, 128, 2, NT, D)
    outs = outs.transpose(0, 1, 3, 4, 2, 5)  # [cores, pair, s, t, p, d]
    return np.ascontiguousarray(outs).reshape(B, H, S, D)


def kernel(query, key, value, attention_mask):
    from concourse.bass_utils import run_bass_kernel_spmd

    in_maps = _pack_inputs(query, key, value, attention_mask)
    nc = _get_program()
    res = run_bass_kernel_spmd(nc, in_maps, core_ids=list(range(N_CORES)))
    return _unpack_output(res.results)
